# revision 4
# baseline (speedup 1.0000x reference)
"""Trainium2 Bass kernel for single-token (decode) multi-head attention.

Problem: q [8,32,1,128], k/v [8,32,4096,128], mask [8,1,1,4096] (fp32)
  out = softmax(q*scale @ k^T + mask) @ v          -> [8,32,1,128]

Sharding: batch across the 8 NeuronCores (B=8 -> 1 batch per core, all 32
heads on-core; no cross-core communication).

The kernel is HBM-bandwidth-bound (~358 GB/s per core). To halve traffic
vs fp32, K/V/q are cast to bf16 on the host before upload (rel err ~4e-3,
well under the 2e-2 gate). bf16 (not fp16) keeps the DVE
scalar_tensor_tensor at full rate: with two non-bf16 SBUF sources the STT
loses its accumulator-readback port and halves throughput.

  - kv row index kv = p*J + j (p = partition, j = row-in-partition).
  - K is staged in HBM as [P, N, J, H] bf16 so a single DMA per 4-head
    group moves [128, G*J*H] with 32 KB contiguous per partition.
  - V gets a ones-column appended (H -> 129) so the AV matmul also
    accumulates sum(exp(scores)) -- the softmax denominator -- into
    psum[0,128]; no separate reduction needed.
  - scores: DVE scalar_tensor_tensor per (head, j): accum over h of
    k*q -> p_raw [128, G*J], then += mask per head.
  - softmax: one ACT exp per 4-head group [128, G*J] -> p_e bf16.
  - AV: PE matmul, p_e column as the 1-wide stationary operand:
    psum[1,129] += p_e[:,i].T @ v'[:, i-block], accumulated over j.
  - normalize: out_row = psum[0:128] * (1/psum[128]) on DVE.
"""

import os

import numpy as np
import ml_dtypes

import concourse.mybir as mybir
import concourse.tile as tile
from concourse import bacc
from concourse.bass_utils import run_bass_kernel_spmd

B, N, T, H, KV = 8, 32, 1, 128, 4096
SCALE = float(H) ** -0.5
P = 128          # partitions
J = KV // P      # 32 kv rows per partition
G = 4            # heads per DMA group
NG = N // G      # number of groups
HV = H + 1       # V width incl. ones column
F32 = mybir.dt.float32
BF16 = mybir.dt.bfloat16
NP_BF16 = np.dtype(ml_dtypes.bfloat16)

_NC_CACHE = None
LAST_RESULT = None  # BassKernelResults of the most recent run (for test harness)


def _build():
    nc = bacc.Bacc()
    q_d = nc.dram_tensor("qb", [P, N * H], BF16, kind="ExternalInput")
    k_d = nc.dram_tensor("kt", [P, N * J * H], BF16, kind="ExternalInput")
    v_d = nc.dram_tensor("vt", [P, N * J * HV], BF16, kind="ExternalInput")
    m_d = nc.dram_tensor("maskr", [P, J], F32, kind="ExternalInput")
    o_d = nc.dram_tensor("out", [1, N * H], F32, kind="ExternalOutput")

    with tile.TileContext(nc) as tc:
        with (
            tc.tile_pool(name="const", bufs=1) as const,
            tc.tile_pool(name="kp", bufs=2) as kp,
            tc.tile_pool(name="vp", bufs=2) as vp,
            tc.tile_pool(name="tmp", bufs=4) as tmpp,
            tc.tile_pool(name="praw", bufs=2) as prp,
            tc.tile_pool(name="pexp", bufs=2) as pep,
            tc.tile_pool(name="po", bufs=8, space="PSUM") as pop,
        ):
            qb = const.tile([P, N * H], BF16)
            nc.sync.dma_start(out=qb[:], in_=q_d[:])
            msk = const.tile([P, J], F32)
            nc.sync.dma_start(out=msk[:], in_=m_d[:])
            out_row = const.tile([1, N * H], F32)
            recip = const.tile([1, N], F32)

            for g in range(NG):
                k_sb = kp.tile([P, G * J * H], BF16)
                nc.sync.dma_start(
                    out=k_sb[:], in_=k_d[:, g * G * J * H:(g + 1) * G * J * H]
                )
                v_sb = vp.tile([P, G * J * HV], BF16)
                nc.scalar.dma_start(
                    out=v_sb[:], in_=v_d[:, g * G * J * HV:(g + 1) * G * J * HV]
                )

                # scores: p_raw[p, n'*J+j] = sum_h k[p, (n'J+j)H+h] * q[n,h]
                p_raw = prp.tile([P, G * J], F32)
                for np_ in range(G):
                    n = g * G + np_
                    for j in range(J):
                        i = np_ * J + j
                        tmp = tmpp.tile([P, H], BF16)
                        nc.vector.scalar_tensor_tensor(
                            out=tmp[:],
                            in0=k_sb[:, i * H:(i + 1) * H],
                            scalar=1.0,
                            in1=qb[:, n * H:(n + 1) * H],
                            op0=mybir.AluOpType.mult,
                            op1=mybir.AluOpType.mult,
                            accum_out=p_raw[:, i:i + 1],
                        )
                    nc.vector.tensor_add(
                        p_raw[:, np_ * J:(np_ + 1) * J],
                        p_raw[:, np_ * J:(np_ + 1) * J],
                        msk[:],
                    )

                # exp for the whole group (denominator comes from ones col)
                p_e = pep.tile([P, G * J], BF16)
                nc.scalar.activation(
                    out=p_e[:],
                    in_=p_raw[:],
                    func=mybir.ActivationFunctionType.Exp,
                )

                # out_unnorm[1, 0:H] and denom [1, H:H+1] via ones column
                for np_ in range(G):
                    n = g * G + np_
                    po = pop.tile([1, HV], F32)
                    for j in range(J):
                        i = np_ * J + j
                        nc.tensor.matmul(
                            po[:],
                            lhsT=p_e[:, i:i + 1],
                            rhs=v_sb[:, i * HV:(i + 1) * HV],
                            start=(j == 0),
                            stop=(j == J - 1),
                        )
                    nc.vector.reciprocal(
                        out=recip[0:1, n:n + 1], in_=po[0:1, H:H + 1]
                    )
                    nc.vector.tensor_scalar_mul(
                        out=out_row[0:1, n * H:(n + 1) * H],
                        in0=po[0:1, 0:H],
                        scalar1=recip[0:1, n:n + 1],
                    )

            nc.sync.dma_start(out=o_d[:], in_=out_row[:])
    nc.finalize()
    return nc


def kernel(q, k, v, mask):
    global _NC_CACHE, LAST_RESULT
    q = np.asarray(q, dtype=np.float32)
    k = np.asarray(k, dtype=np.float32)
    v = np.asarray(v, dtype=np.float32)
    mask = np.asarray(mask, dtype=np.float32)

    if _NC_CACHE is None:
        _NC_CACHE = _build()
    nc = _NC_CACHE

    # host-side restaging: bf16 cast + partition-major layout
    #   kt/vt: [N, KV, H] -> [N, P, J, H] -> [P, N, J, H] (kv = p*J + j)
    kt = k.reshape(B, N, P, J, H).transpose(0, 2, 1, 3, 4)
    kt = np.ascontiguousarray(kt).astype(NP_BF16).reshape(B, P, N * J * H)
    vt = v.reshape(B, N, P, J, H).transpose(0, 2, 1, 3, 4)
    ones = np.ones((B, P, N, J, 1), dtype=np.float32)
    vt = np.concatenate([vt, ones], axis=-1)
    vt = np.ascontiguousarray(vt).astype(NP_BF16).reshape(B, P, N * J * HV)
    qs = (q[:, :, 0, :] * SCALE).astype(NP_BF16).reshape(B, 1, N * H)
    qb = np.broadcast_to(qs, (B, P, N * H))

    in_maps = []
    for b in range(B):
        in_maps.append({
            "qb": np.ascontiguousarray(qb[b]),
            "kt": kt[b],
            "vt": vt[b],
            "maskr": np.ascontiguousarray(mask[b, 0, 0, :].reshape(P, J)),
        })

    res = run_bass_kernel_spmd(
        nc,
        in_maps,
        core_ids=list(range(B)),
        trace=bool(int(os.environ.get("KERNEL_TRACE", "0"))),
    )
    LAST_RESULT = res
    out = np.stack([r["out"].reshape(N, H) for r in res.results])
    return out[:, :, None, :].astype(np.float32)
